# revision 44
# baseline (speedup 1.0000x reference)
"""NonLocalAttention (embedded gaussian, no softmax) on 8 trn2 NeuronCores.

Reference math (per sample, all linear — no softmax):
    theta = conv1x1(a, theta_w, theta_b)        # [Ci, N]
    phi   = conv1x1(b, phi_w, phi_b)            # [Ci, N]
    g     = conv1x1(b, g_w, g_b)                # [Ci, N]
    f     = theta^T @ phi / N                   # [N, N]
    y     = f @ g^T                             # [N, Ci]
    out   = BN(W_w @ y^T)                       # [C, N]

Everything is linear, so the whole network collapses to a per-sample
256x256 Gram matrix of b plus small weight products:
    S   = b b^T                                   # [256, 256], symmetric
    M3  = S K2,  K2 = g_w^T (bn_scale*W_w)^T      # K2 host-precomputed
    R^T = phi_w M3                                # [128, 256]
    ta  = theta'^T-contract a  (theta' = theta_w/N)
    out = R^T-contract ta + shift

S accumulates across 128-pixel chunks of the HOST-TRANSPOSED b (pixels on
partitions), matmul(lhsT=bT[:, c-half], rhs=bT): no PE transposes, no
per-chunk evictions — phase 1 is pure PE. M3 = S K2 needs S^T tiles as
stationary operands, which by symmetry are just the stored S tiles.

All activations and weights move as bf16 (halves HBM traffic), f32 PSUM
accumulation; output returns bf16 and is cast to f32 on the host. Biases
(zero in this problem, but handled exactly): phi_b/g_b fold into a
host-computed correction to R^T (needs only rowsums of b); theta_b is
the bias of the ta eviction; BN scale folds into K2 on the host.

Sharding: 8 cores = 4 samples x 2 pixel-halves of `a`. Each core loads
the full per-sample b (S is duplicated across the pair — cheaper than
any cross-core exchange) and its half of a; no inter-core communication.
"""

import numpy as np

B, C, Ci, H, W = 4, 256, 128, 64, 64
N_PIX = H * W            # 4096 pixels per sample
N_CORES = 8
HALF = N_PIX // 2        # 2048 output pixels per core
P = 128
CC = C // P              # 2 channel chunks
B_CHUNKS = (768, 512, 512, 512, 512, 512, 512, 256)  # pixels per b DMA
MCH = N_PIX // P         # 32 pixel chunks for the S accumulation
NTA = 4                  # a DMA chunks == ta/out blocks
RB = 512                 # output row block
BN_EPS = 1e-5

WARMUP_MM = 18           # junk matmuls to lift the PE HAM throttle early

# wpack column layout (bf16, partition dim = 128):
#   [0,256)     phi_w^T    2 halves of [c1_half, ci1]
#   [256,768)   K2         2 halves of [c2_half, c_out(256)]
#   [768,1024)  (theta_w/N)^T  2 halves of [c_half, ci1]
WCOLS = 1024
# vpack (f32): [0,2) bn shift per cc, [2] theta_b/N, [3,259) R^T bias corr
VCOLS = 259

_CACHE = {}


def _build():
    import concourse.bacc as bacc
    import concourse.mybir as mybir
    import concourse.tile as tile
    from concourse.masks import make_identity

    f32 = mybir.dt.float32
    bf16 = mybir.dt.bfloat16
    Act = mybir.ActivationFunctionType

    nc = bacc.Bacc("TRN2", num_devices=N_CORES)

    b_d = nc.dram_tensor("bT", [N_PIX, C], bf16, kind="ExternalInput")
    wpack_d = nc.dram_tensor("wpack", [P, WCOLS], bf16, kind="ExternalInput")
    vpack_d = nc.dram_tensor("vpack", [P, VCOLS], f32, kind="ExternalInput")
    a_d = nc.dram_tensor("a_half", [NTA, CC, P, HALF // NTA], bf16,
                         kind="ExternalInput")
    out_d = nc.dram_tensor("out", [CC, P, HALF], bf16, kind="ExternalOutput")

    with tile.TileContext(nc) as tc:
        with (
            tc.tile_pool(name="const", bufs=1) as cpool,
            tc.tile_pool(name="big", bufs=1) as bpool,
            tc.tile_pool(name="work", bufs=2) as wpool,
            tc.tile_pool(name="ps", bufs=4, space="PSUM") as ppool,
        ):
            bt_sb = bpool.tile([P, MCH, C], bf16)
            wpack_sb = cpool.tile([P, WCOLS], bf16)
            vpack_sb = cpool.tile([P, VCOLS], f32)
            a_sb = bpool.tile([P, CC, HALF], bf16)

            phwT = wpack_sb[:, 0:256].rearrange("p (h k) -> p h k", h=2)
            k2_sb = wpack_sb[:, 256:768].rearrange("p (h k) -> p h k", h=2)
            thwT = wpack_sb[:, 768:1024].rearrange("p (h k) -> p h k", h=2)
            shift_in = vpack_sb[:, 0:2]
            thb_sb = vpack_sb[:, 2:3]
            rtc_sb = vpack_sb[:, 3:259]

            # single SP FIFO: bT chunks first (phase 1 streams them), then
            # weights (needed mid-kernel), then a chunks (tail input).
            pos = 0
            for sz in B_CHUNKS:
                k0, k1 = pos // P, (pos + sz) // P
                nc.sync.dma_start(
                    out=bt_sb[:, k0:k1, :],
                    in_=b_d[pos : pos + sz, :].rearrange(
                        "(k p) c -> p k c", p=P),
                )
                pos += sz
            assert pos == N_PIX
            nc.sync.dma_start(out=wpack_sb[:], in_=wpack_d[:])
            nc.sync.dma_start(out=vpack_sb[:], in_=vpack_d[:])
            ap = HALF // NTA
            for t in range(NTA):
                nc.sync.dma_start(
                    out=a_sb[:, :, t * ap : (t + 1) * ap],
                    in_=a_d[t].rearrange("c p x -> p c x"),
                )

            # ---- engine warmup ------------------------------------------
            # Touch the scalar engine immediately so its activation-table
            # load (1.3us) runs during the initial DMA wait, not in front of
            # the first eviction.
            act_warm = cpool.tile([P, 8], f32)
            nc.scalar.memzero(act_warm[:, 0:4])
            nc.scalar.copy(act_warm[:, 4:8], act_warm[:, 0:4])

            # PE warmup: sustained matmuls on a gpsimd-built tile so the HAM
            # clock gate lifts before the real work arrives (no DMA needed).
            if True:
                ident_f32 = cpool.tile([P, P], f32)
                ident_bf = cpool.tile([P, P], bf16)
                make_identity(nc, ident_f32[:])
                nc.vector.tensor_copy(ident_bf[:], ident_f32[:])
                warm_ps = ppool.tile([P, P], f32, tag="warm", bufs=1,
                                     name="warm_ps")
                for i in range(WARMUP_MM):
                    nc.tensor.matmul(
                        warm_ps[:], ident_bf[:], ident_bf[:],
                        start=True, stop=True,
                    )

            # ---- phase 1: S = b b^T, accumulated in PSUM ------------------
            # S is symmetric: compute only blocks S00, S10, S11 (3 matmuls
            # per chunk instead of 4); S01 = S10^T via one PE transpose.
            s00_ps = ppool.tile([P, P], f32, tag="acc", bufs=3, name="s00_ps")
            s10_ps = ppool.tile([P, P], f32, tag="acc", bufs=3, name="s10_ps")
            s11_ps = ppool.tile([P, P], f32, tag="acc", bufs=3, name="s11_ps")
            for m in range(MCH):
                bt0 = bt_sb[:, m, 0:P]
                bt1 = bt_sb[:, m, P:C]
                st = (m == 0)
                sp = (m == MCH - 1)
                nc.tensor.matmul(s00_ps[:], bt0, bt0, start=st, stop=sp)
                nc.tensor.matmul(s10_ps[:], bt1, bt0, start=st, stop=sp)
                nc.tensor.matmul(s11_ps[:], bt1, bt1, start=st, stop=sp)
            s00_sb = bpool.tile([P, P], bf16)
            s10_sb = bpool.tile([P, P], bf16)
            s11_sb = bpool.tile([P, P], bf16)
            s01_sb = bpool.tile([P, P], bf16)
            nc.vector.tensor_copy(s00_sb[:], s00_ps[:])
            nc.scalar.copy(s10_sb[:], s10_ps[:])
            nc.vector.tensor_copy(s11_sb[:], s11_ps[:])
            tp_ps = ppool.tile([P, P], bf16, tag="ps", name="tp_ps")
            nc.tensor.transpose(tp_ps[:], s10_sb[:], ident_bf[:])
            nc.scalar.copy(s01_sb[:], tp_ps[:])
            # lhsT block for (c1_half, c2_half): S[c2 in hc2, c1 in hc1]
            s_blk = {(0, 0): s00_sb, (0, 1): s10_sb,
                     (1, 0): s01_sb, (1, 1): s11_sb}

            # ---- tail chain: M3 = S K2, R^T = phi_w M3 --------------------
            NBLK = HALF // RB
            ta_sb = bpool.tile([Ci, HALF], bf16)
            rt_sb = bpool.tile([Ci, C], bf16)

            def emit_ta(t, evict=True):
                rows = slice(t * RB, (t + 1) * RB)
                ta_ps = ppool.tile([Ci, RB], f32, tag="ps", name=f"taps{t}")
                for cc in range(CC):
                    nc.tensor.matmul(ta_ps[:], thwT[:, cc, :],
                                     a_sb[:, cc, rows],
                                     start=(cc == 0), stop=(cc == CC - 1))
                if evict:
                    emit_ta_evict(t, ta_ps)
                return ta_ps

            def emit_ta_evict(t, ta_ps):
                rows = slice(t * RB, (t + 1) * RB)
                if t < 3:
                    nc.scalar.activation(ta_sb[:, rows], ta_ps[:],
                                         Act.Identity, bias=thb_sb)
                else:
                    nc.vector.tensor_tensor(
                        ta_sb[:, rows], ta_ps[:],
                        thb_sb.broadcast_to([Ci, RB]),
                        op=mybir.AluOpType.add)

            # M3[c1, co] = sum_c2 S[c1, c2] K2[c2, co]; stationary operands
            # come from the symmetric block set.
            m3_sbs = []
            for hc1 in range(2):
                m3_ps = ppool.tile([P, C], f32, tag="ps", name=f"m3ps{hc1}")
                for hc2 in range(2):
                    nc.tensor.matmul(
                        m3_ps[:], s_blk[(hc1, hc2)][:],
                        k2_sb[:, hc2, :],
                        start=(hc2 == 0), stop=(hc2 == 1),
                    )
                m3_sb = bpool.tile([P, C], bf16, name=f"m3sb{hc1}")
                if hc1 == 0:
                    nc.vector.tensor_copy(m3_sb[:], m3_ps[:])
                else:
                    nc.scalar.copy(m3_sb[:], m3_ps[:])
                m3_sbs.append(m3_sb)
                if hc1 == 0:
                    emit_ta(0)
            emit_ta(1)
            rt_ps = ppool.tile([Ci, C], f32, tag="ps", name="rt_ps")
            for h in range(2):
                nc.tensor.matmul(rt_ps[:], phwT[:, h, :], m3_sbs[h][:],
                                 start=(h == 0), stop=(h == 1))
            nc.vector.tensor_tensor(rt_sb[:], rt_ps[:], rtc_sb,
                                    op=mybir.AluOpType.add)
            emit_ta(2)
            # ta3's matmuls run now, but its eviction is emitted after the
            # first output block: the osb r0 evictions must win both engines
            # the moment out-r0 finishes, because the store wire start sets
            # the kernel end while out-r3 has slack.
            ta3_ps = emit_ta(3, evict=False)

            # ---- out = R^T-contract ta, BN shift, store -------------------
            for r in range(NBLK):
                if r == 1:
                    emit_ta_evict(3, ta3_ps)
                rows = slice(r * RB, (r + 1) * RB)
                osb = wpool.tile([P, CC, RB], bf16, tag="osb", bufs=4,
                                 name=f"osb{r}")
                for co in range(CC):
                    o_ps = ppool.tile([P, RB], f32, tag="ps",
                                      name=f"ops{r}{co}")
                    nc.tensor.matmul(o_ps[:], rt_sb[:, co * P : (co + 1) * P],
                                     ta_sb[:, rows], start=True, stop=True)
                    if co == 0:
                        nc.scalar.activation(osb[:, 0, :], o_ps[:],
                                             Act.Identity,
                                             bias=shift_in[:, 0:1])
                    else:
                        nc.vector.tensor_tensor(
                            osb[:, 1, :], o_ps[:],
                            shift_in[:, 1:2].broadcast_to([P, RB]),
                            op=mybir.AluOpType.add,
                        )
                nc.sync.dma_start(
                    out=out_d[:, :, rows].rearrange("c p r -> p c r"),
                    in_=osb[:],
                )

    nc.compile()
    return nc


def _get_nc():
    if "nc" not in _CACHE:
        _CACHE["nc"] = _build()
    return _CACHE["nc"]


def _prep_in_maps(a, b, theta_w, theta_b, phi_w, phi_b, g_w, g_b, W_w,
                  bn_gamma, bn_beta, bn_mean, bn_var):
    import ml_dtypes

    f = np.float32
    bf = ml_dtypes.bfloat16
    a4 = np.asarray(a, f).reshape(B, C, N_PIX)
    b4 = np.asarray(b, f).reshape(B, C, N_PIX)
    theta_w = np.asarray(theta_w, f)
    phi_w = np.asarray(phi_w, f)
    g_w = np.asarray(g_w, f)
    W_w = np.asarray(W_w, f)
    theta_b = np.asarray(theta_b, f)
    phi_b = np.asarray(phi_b, f)
    g_b = np.asarray(g_b, f)

    scale = (np.asarray(bn_gamma, f)
             / np.sqrt(np.asarray(bn_var, f) + BN_EPS)).astype(f)
    shift = (np.asarray(bn_beta, f) - np.asarray(bn_mean, f) * scale).astype(f)
    inv_n = 1.0 / np.float64(N_PIX)
    WT = (W_w * scale[:, None]).T                # [ci2, c_out]

    wpack = np.zeros((P, WCOLS), f)
    wpack[:, 0:128] = phi_w.T[0:P]
    wpack[:, 128:256] = phi_w.T[P:C]
    K2 = g_w.T @ WT                              # [c2, c_out]
    wpack[:, 256:512] = K2[0:P]
    wpack[:, 512:768] = K2[P:C]
    thT = (theta_w * inv_n).T                    # [C, Ci]
    wpack[:, 768:896] = thT[0:P]
    wpack[:, 896:1024] = thT[P:C]
    wpack = np.ascontiguousarray(wpack.astype(bf))

    # R^T bias correction from rowsums of b (exact; zero for zero biases)
    rsb = b4.sum(axis=2)                        # [B, C]
    s_phi = rsb @ phi_w.T                       # [B, Ci]
    s_g = rsb @ g_w.T                           # [B, Ci]
    qp = HALF // NTA

    in_maps = []
    for core in range(N_CORES):
        s, h = divmod(core, 2)
        cmi = (phi_b[:, None] * s_g[s][None, :]
               + s_phi[s][:, None] * g_b[None, :]
               + N_PIX * phi_b[:, None] * g_b[None, :]).astype(f)
        rtc = cmi @ WT                          # [ci1, c_out]
        vpack = np.zeros((P, VCOLS), f)
        vpack[:, 0] = shift[:P]
        vpack[:, 1] = shift[P:]
        vpack[:, 2] = theta_b * inv_n
        vpack[:, 3:259] = rtc
        ah = a4[s][:, h * HALF : (h + 1) * HALF]
        in_maps.append({
            "bT": np.ascontiguousarray(b4[s].T.astype(bf)),
            "wpack": wpack,
            "vpack": np.ascontiguousarray(vpack),
            "a_half": np.ascontiguousarray(
                ah.reshape(CC, P, NTA, qp).transpose(2, 0, 1, 3).astype(bf)),
        })
    return in_maps


def run(inputs: dict, trace: bool = False):
    from concourse.bass_utils import run_bass_kernel_spmd

    nc = _get_nc()
    in_maps = _prep_in_maps(**inputs)
    res = run_bass_kernel_spmd(nc, in_maps, list(range(N_CORES)), trace=trace)
    out = np.empty((B, C, N_PIX), np.float32)
    for core in range(N_CORES):
        s, h = divmod(core, 2)
        out[s][:, h * HALF : (h + 1) * HALF] = \
            res.results[core]["out"].reshape(C, HALF).astype(np.float32)
    return out.reshape(B, C, H, W), res


def kernel(**inputs) -> np.ndarray:
    out, _ = run(inputs, trace=False)
    return out


# revision 47
# speedup vs baseline: 1.0051x; 1.0051x over previous
"""NonLocalAttention (embedded gaussian, no softmax) on 8 trn2 NeuronCores.

Reference math (per sample, all linear — no softmax):
    theta = conv1x1(a, theta_w, theta_b)        # [Ci, N]
    phi   = conv1x1(b, phi_w, phi_b)            # [Ci, N]
    g     = conv1x1(b, g_w, g_b)                # [Ci, N]
    f     = theta^T @ phi / N                   # [N, N]
    y     = f @ g^T                             # [N, Ci]
    out   = BN(W_w @ y^T)                       # [C, N]

Everything is linear, so the whole network collapses to a per-sample
256x256 Gram matrix of b plus small weight products:
    S   = b b^T                                   # [256, 256], symmetric
    M3  = S K2,  K2 = g_w^T (bn_scale*W_w)^T      # K2 host-precomputed
    GT  = K4 M3, K4 = (theta_w/N)^T phi_w         # K4 host-precomputed
    out = GT-contract a + shift                   # a consumed straight
                                                  # from DMA, no ta stage

S accumulates across 128-pixel chunks of the HOST-TRANSPOSED b (pixels on
partitions), matmul(lhsT=bT[:, c-half], rhs=bT): no PE transposes, no
per-chunk evictions — phase 1 is pure PE. M3 = S K2 needs S^T tiles as
stationary operands, which by symmetry are just the stored S tiles.

All activations and weights move as bf16 (halves HBM traffic), f32 PSUM
accumulation; output returns bf16 and is cast to f32 on the host. Biases
(zero in this problem, but handled exactly): phi_b/g_b fold into a
host-computed correction to R^T (needs only rowsums of b); theta_b is
the bias of the ta eviction; BN scale folds into K2 on the host.

Sharding: 8 cores = 4 samples x 2 pixel-halves of `a`. Each core loads
the full per-sample b (S is duplicated across the pair — cheaper than
any cross-core exchange) and its half of a; no inter-core communication.
"""

import numpy as np

B, C, Ci, H, W = 4, 256, 128, 64, 64
N_PIX = H * W            # 4096 pixels per sample
N_CORES = 8
HALF = N_PIX // 2        # 2048 output pixels per core
P = 128
CC = C // P              # 2 channel chunks
B_CHUNKS = (768, 512, 512, 512, 512, 512, 512, 256)  # pixels per b DMA
MCH = N_PIX // P         # 32 pixel chunks for the S accumulation
NTA = 4                  # a DMA chunks == ta/out blocks
RB = 512                 # output row block
BN_EPS = 1e-5

WARMUP_MM = 18           # junk matmuls to lift the PE HAM throttle early

# wpack column layout (bf16, partition dim = 128):
#   [0,512)     K4^T   2 halves of [c1_half, c(256)]
#   [512,1024)  K2     2 halves of [c2_half, c_out(256)]
#   [1024,1280) (bn_scale*W_w)^T [ci2, c_out]  (for the bias correction)
WCOLS = 1280
# vpack (f32): [0,2) bn shift per cc
VCOLS = 2
# hcpack (bf16): [ci2, c] per-core bias-correction operand (theta'^T cmi)^T
HCOLS = 256

_CACHE = {}


def _build():
    import concourse.bacc as bacc
    import concourse.mybir as mybir
    import concourse.tile as tile
    from concourse.masks import make_identity

    f32 = mybir.dt.float32
    bf16 = mybir.dt.bfloat16
    Act = mybir.ActivationFunctionType

    nc = bacc.Bacc("TRN2", num_devices=N_CORES)

    b_d = nc.dram_tensor("bT", [N_PIX, C], bf16, kind="ExternalInput")
    wpack_d = nc.dram_tensor("wpack", [P, WCOLS], bf16, kind="ExternalInput")
    vpack_d = nc.dram_tensor("vpack", [P, VCOLS], f32, kind="ExternalInput")
    hc_d = nc.dram_tensor("hcpack", [P, HCOLS], bf16, kind="ExternalInput")
    a_d = nc.dram_tensor("a_half", [NTA, CC, P, HALF // NTA], bf16,
                         kind="ExternalInput")
    out_d = nc.dram_tensor("out", [CC, P, HALF], bf16, kind="ExternalOutput")

    with tile.TileContext(nc) as tc:
        with (
            tc.tile_pool(name="const", bufs=1) as cpool,
            tc.tile_pool(name="big", bufs=1) as bpool,
            tc.tile_pool(name="work", bufs=2) as wpool,
            tc.tile_pool(name="ps", bufs=4, space="PSUM") as ppool,
        ):
            bt_sb = bpool.tile([P, MCH, C], bf16)
            wpack_sb = cpool.tile([P, WCOLS], bf16)
            vpack_sb = cpool.tile([P, VCOLS], f32)
            hc_sb = cpool.tile([P, HCOLS], bf16)
            a_sb = bpool.tile([P, CC, HALF], bf16)

            k4T = wpack_sb[:, 0:512].rearrange("p (h k) -> p h k", h=2)
            k2_sb = wpack_sb[:, 512:1024].rearrange("p (h k) -> p h k", h=2)
            WT_sb = wpack_sb[:, 1024:1280]
            shift_in = vpack_sb[:, 0:2]

            # single SP FIFO: bT chunks first (phase 1 streams them), then
            # weights (needed mid-kernel), then a chunks (tail input).
            pos = 0
            for sz in B_CHUNKS:
                k0, k1 = pos // P, (pos + sz) // P
                nc.sync.dma_start(
                    out=bt_sb[:, k0:k1, :],
                    in_=b_d[pos : pos + sz, :].rearrange(
                        "(k p) c -> p k c", p=P),
                )
                pos += sz
            assert pos == N_PIX
            nc.sync.dma_start(out=wpack_sb[:], in_=wpack_d[:])
            nc.sync.dma_start(out=vpack_sb[:], in_=vpack_d[:])
            nc.sync.dma_start(out=hc_sb[:], in_=hc_d[:])
            ap = HALF // NTA
            for t in range(NTA):
                nc.sync.dma_start(
                    out=a_sb[:, :, t * ap : (t + 1) * ap],
                    in_=a_d[t].rearrange("c p x -> p c x"),
                )

            # ---- engine warmup ------------------------------------------
            # Touch the scalar engine immediately so its activation-table
            # load (1.3us) runs during the initial DMA wait, not in front of
            # the first eviction.
            act_warm = cpool.tile([P, 8], f32)
            nc.scalar.memzero(act_warm[:, 0:4])
            nc.scalar.copy(act_warm[:, 4:8], act_warm[:, 0:4])

            # PE warmup: sustained matmuls on a gpsimd-built tile so the HAM
            # clock gate lifts before the real work arrives (no DMA needed).
            if True:
                ident_f32 = cpool.tile([P, P], f32)
                ident_bf = cpool.tile([P, P], bf16)
                make_identity(nc, ident_f32[:])
                nc.vector.tensor_copy(ident_bf[:], ident_f32[:])
                warm_ps = ppool.tile([P, P], f32, tag="warm", bufs=1,
                                     name="warm_ps")
                for i in range(WARMUP_MM):
                    nc.tensor.matmul(
                        warm_ps[:], ident_bf[:], ident_bf[:],
                        start=True, stop=True,
                    )

            # ---- phase 1: S = b b^T, accumulated in PSUM ------------------
            # S is symmetric: compute only blocks S00, S10, S11 (3 matmuls
            # per chunk instead of 4); S01 = S10^T via one PE transpose.
            s00_ps = ppool.tile([P, P], f32, tag="acc", bufs=3, name="s00_ps")
            s10_ps = ppool.tile([P, P], f32, tag="acc", bufs=3, name="s10_ps")
            s11_ps = ppool.tile([P, P], f32, tag="acc", bufs=3, name="s11_ps")
            for m in range(MCH):
                bt0 = bt_sb[:, m, 0:P]
                bt1 = bt_sb[:, m, P:C]
                st = (m == 0)
                sp = (m == MCH - 1)
                nc.tensor.matmul(s00_ps[:], bt0, bt0, start=st, stop=sp)
                nc.tensor.matmul(s10_ps[:], bt1, bt0, start=st, stop=sp)
                nc.tensor.matmul(s11_ps[:], bt1, bt1, start=st, stop=sp)
            s00_sb = bpool.tile([P, P], bf16)
            s10_sb = bpool.tile([P, P], bf16)
            s11_sb = bpool.tile([P, P], bf16)
            s01_sb = bpool.tile([P, P], bf16)
            nc.vector.tensor_copy(s00_sb[:], s00_ps[:])
            nc.scalar.copy(s10_sb[:], s10_ps[:])
            nc.vector.tensor_copy(s11_sb[:], s11_ps[:])
            tp_ps = ppool.tile([P, P], bf16, tag="ps", name="tp_ps")
            nc.tensor.transpose(tp_ps[:], s10_sb[:], ident_bf[:])
            nc.scalar.copy(s01_sb[:], tp_ps[:])
            # lhsT block for (c1_half, c2_half): S[c2 in hc2, c1 in hc1]
            s_blk = {(0, 0): s00_sb, (0, 1): s10_sb,
                     (1, 0): s01_sb, (1, 1): s11_sb}

            # ---- tail chain: M3 = S K2, GT = K4 M3 (+ bias corr) ----------
            NBLK = HALF // RB

            # M3[c1, co] = sum_c2 S[c1, c2] K2[c2, co]; stationary operands
            # come from the symmetric block set.
            m3_sbs = []
            for hc1 in range(2):
                m3_ps = ppool.tile([P, C], f32, tag="ps", name=f"m3ps{hc1}")
                for hc2 in range(2):
                    nc.tensor.matmul(
                        m3_ps[:], s_blk[(hc1, hc2)][:],
                        k2_sb[:, hc2, :],
                        start=(hc2 == 0), stop=(hc2 == 1),
                    )
                m3_sb = bpool.tile([P, C], bf16, name=f"m3sb{hc1}")
                if hc1 == 0:
                    nc.vector.tensor_copy(m3_sb[:], m3_ps[:])
                else:
                    nc.scalar.copy(m3_sb[:], m3_ps[:])
                m3_sbs.append(m3_sb)

            # GT[c, co] = sum_c1 K4[c, c1] M3[c1, co] + bias correction
            # (hc^T-contract W^T, exact, zero when the conv biases are zero)
            gt_sbs = []
            for cs in range(2):
                gt_ps = ppool.tile([P, C], f32, tag="ps", name=f"gtps{cs}")
                nc.tensor.matmul(gt_ps[:], hc_sb[:, cs * P : (cs + 1) * P],
                                 WT_sb[:], start=True, stop=False)
                for h in range(2):
                    nc.tensor.matmul(
                        gt_ps[:], k4T[:, h, cs * P : (cs + 1) * P],
                        m3_sbs[h][:], start=False, stop=(h == 1),
                    )
                gt_sb = bpool.tile([P, C], bf16, name=f"gtsb{cs}")
                if cs == 0:
                    nc.vector.tensor_copy(gt_sb[:], gt_ps[:])
                else:
                    nc.scalar.copy(gt_sb[:], gt_ps[:])
                gt_sbs.append(gt_sb)

            # ---- out = GT-contract a, BN shift, store ---------------------
            for r in range(NBLK):
                rows = slice(r * RB, (r + 1) * RB)
                osb = wpool.tile([P, CC, RB], bf16, tag="osb", bufs=4,
                                 name=f"osb{r}")
                for co in range(CC):
                    o_ps = ppool.tile([P, RB], f32, tag="ps",
                                      name=f"ops{r}{co}")
                    for cc in range(CC):
                        nc.tensor.matmul(
                            o_ps[:], gt_sbs[cc][:, co * P : (co + 1) * P],
                            a_sb[:, cc, rows],
                            start=(cc == 0), stop=(cc == CC - 1),
                        )
                    if co == 0:
                        nc.scalar.activation(osb[:, 0, :], o_ps[:],
                                             Act.Identity,
                                             bias=shift_in[:, 0:1])
                    else:
                        nc.vector.tensor_tensor(
                            osb[:, 1, :], o_ps[:],
                            shift_in[:, 1:2].broadcast_to([P, RB]),
                            op=mybir.AluOpType.add,
                        )
                nc.sync.dma_start(
                    out=out_d[:, :, rows].rearrange("c p r -> p c r"),
                    in_=osb[:],
                )

    nc.compile()
    return nc


def _get_nc():
    if "nc" not in _CACHE:
        _CACHE["nc"] = _build()
    return _CACHE["nc"]


def _prep_in_maps(a, b, theta_w, theta_b, phi_w, phi_b, g_w, g_b, W_w,
                  bn_gamma, bn_beta, bn_mean, bn_var):
    import ml_dtypes

    f = np.float32
    bf = ml_dtypes.bfloat16
    a4 = np.asarray(a, f).reshape(B, C, N_PIX)
    b4 = np.asarray(b, f).reshape(B, C, N_PIX)
    theta_w = np.asarray(theta_w, f)
    phi_w = np.asarray(phi_w, f)
    g_w = np.asarray(g_w, f)
    W_w = np.asarray(W_w, f)
    theta_b = np.asarray(theta_b, f)
    phi_b = np.asarray(phi_b, f)
    g_b = np.asarray(g_b, f)

    scale = (np.asarray(bn_gamma, f)
             / np.sqrt(np.asarray(bn_var, f) + BN_EPS)).astype(f)
    shift = (np.asarray(bn_beta, f) - np.asarray(bn_mean, f) * scale).astype(f)
    inv_n = 1.0 / np.float64(N_PIX)
    WT = (W_w * scale[:, None]).T                # [ci2, c_out]

    wpack = np.zeros((P, WCOLS), f)
    thT = (theta_w * inv_n).T                    # [C, Ci]
    K4 = thT @ phi_w                             # [c, c1]
    wpack[:, 0:256] = K4.T[0:P]
    wpack[:, 256:512] = K4.T[P:C]
    K2 = g_w.T @ WT                              # [c2, c_out]
    wpack[:, 512:768] = K2[0:P]
    wpack[:, 768:1024] = K2[P:C]
    wpack[:, 1024:1280] = WT
    wpack = np.ascontiguousarray(wpack.astype(bf))

    # theta_b folded into `a` as a per-channel offset x with
    # (theta_w/N) x = theta_b/N  (exact for full-row-rank theta_w; x = 0
    # when theta_b = 0, which also covers any rank deficiency there)
    A = theta_w * inv_n
    if np.any(theta_b):
        x = np.linalg.lstsq(A, theta_b * np.float64(inv_n), rcond=None)[0]
        a4 = a4 + x.astype(f)[None, :, None]

    # bias correction from rowsums of b (exact; zero for zero biases)
    rsb = b4.sum(axis=2)                        # [B, C]
    s_phi = rsb @ phi_w.T                       # [B, Ci]
    s_g = rsb @ g_w.T                           # [B, Ci]
    qp = HALF // NTA

    in_maps = []
    for core in range(N_CORES):
        s, h = divmod(core, 2)
        cmi = (phi_b[:, None] * s_g[s][None, :]
               + s_phi[s][:, None] * g_b[None, :]
               + N_PIX * phi_b[:, None] * g_b[None, :]).astype(f)
        hc = thT @ cmi                          # [c, ci2]
        vpack = np.zeros((P, VCOLS), f)
        vpack[:, 0] = shift[:P]
        vpack[:, 1] = shift[P:]
        ah = a4[s][:, h * HALF : (h + 1) * HALF]
        in_maps.append({
            "bT": np.ascontiguousarray(b4[s].T.astype(bf)),
            "wpack": wpack,
            "vpack": np.ascontiguousarray(vpack),
            "hcpack": np.ascontiguousarray(hc.T.astype(bf)),
            "a_half": np.ascontiguousarray(
                ah.reshape(CC, P, NTA, qp).transpose(2, 0, 1, 3).astype(bf)),
        })
    return in_maps


def run(inputs: dict, trace: bool = False):
    from concourse.bass_utils import run_bass_kernel_spmd

    nc = _get_nc()
    in_maps = _prep_in_maps(**inputs)
    res = run_bass_kernel_spmd(nc, in_maps, list(range(N_CORES)), trace=trace)
    out = np.empty((B, C, N_PIX), np.float32)
    for core in range(N_CORES):
        s, h = divmod(core, 2)
        out[s][:, h * HALF : (h + 1) * HALF] = \
            res.results[core]["out"].reshape(C, HALF).astype(np.float32)
    return out.reshape(B, C, H, W), res


def kernel(**inputs) -> np.ndarray:
    out, _ = run(inputs, trace=False)
    return out
